# revision 14
# baseline (speedup 1.0000x reference)
"""Contrastive loss (GRACE-style) on 8 Trainium2 NeuronCores — fp8 edition.

loss = sum_i 0.5*(l1_i + l2_i)
  l1 = -log(diag(exp(h1@h2.T/t)) / (rowsum(exp(h1@h1.T/t)) + rowsum(exp(h1@h2.T/t)) - diag(exp(h1@h1.T/t))))
  l2 = same with h1<->h2;  h = z / ||z||_row,  t = 0.2

Sharding: columns (j) of the similarity matrices are sharded across 8 cores
(each core owns a 1024-column chunk of both h1 and h2). Each core computes,
for ALL 8192 rows i, the partial sums over its j-chunk of
exp(s_i * (z_i . h_j)), where the row normalization 1/(16*tau*||z_i||) is a
per-partition ACT scale. All matmuls run fp8(e4m3) DoubleRow (2x PE rate):
stationary = raw z.T tiles, moving = (h*16).T chunk tiles, contraction pairs
along the kd dimension. Per row-block: exp+rowsum of refl and between tiles
(ACT exp -> DVE reduce), plus ones-matmul colsums of exp'd between tiles
(= between.T rowsums for own chunk rows, PSUM-accumulated over all 64 row
blocks). Host (numpy, O(N*D)) prepares fp8 inputs/scales and applies the
exact diagonal corrections + logs in float64.
"""

import numpy as np
import ml_dtypes

N = 8192
D = 512
NCORES = 8
CH = N // NCORES  # 1024 columns per core
P = 128
KD = D // P  # 4 k-subtiles; DoubleRow consumes them in pairs
NIB = N // P  # 64 row blocks
TAU = 0.2
RS = 16.0  # rhs pre-scale to keep fp8 h values in the normal range

_CACHE = {}


def _build(repeat=1, loop=None):
    import concourse.tile as tile
    from concourse import bacc, mybir

    f32 = mybir.dt.float32
    bf16 = mybir.dt.bfloat16
    fp8 = mybir.dt.float8e4
    AF = mybir.ActivationFunctionType
    ALU = mybir.AluOpType
    DR = mybir.MatmulPerfMode.DoubleRow

    nc = bacc.Bacc("TRN2", target_bir_lowering=False, debug=False,
                   num_devices=NCORES)

    zt1 = nc.dram_tensor("zt1", [D, N], fp8, kind="ExternalInput")
    zt2 = nc.dram_tensor("zt2", [D, N], fp8, kind="ExternalInput")
    rh1 = nc.dram_tensor("rh1", [D, CH], fp8, kind="ExternalInput")
    rh2 = nc.dram_tensor("rh2", [D, CH], fp8, kind="ExternalInput")
    s1 = nc.dram_tensor("s1", [N], f32, kind="ExternalInput")
    s2 = nc.dram_tensor("s2", [N], f32, kind="ExternalInput")
    partials = nc.dram_tensor("partials", [2, N], f32, kind="ExternalOutput")
    ecol = nc.dram_tensor("ecol", [CH], f32, kind="ExternalOutput")

    zt1v = zt1.rearrange("(k p) n -> p k n", p=P)
    zt2v = zt2.rearrange("(k p) n -> p k n", p=P)
    rh1v = rh1.rearrange("(k p) n -> p k n", p=P)
    rh2v = rh2.rearrange("(k p) n -> p k n", p=P)

    with tile.TileContext(nc) as tc:
        with (
            tc.tile_pool(name="singles", bufs=1) as singles,
            tc.tile_pool(name="es", bufs=2) as esp,
            tc.tile_pool(name="ps", bufs=3, space="PSUM") as psp,
            tc.tile_pool(name="pscol", bufs=1, space="PSUM") as pscolp,
        ):
            # ---- persistent buffers ----
            zt1s = singles.tile([P, KD, N], fp8, tag="zt1s")
            zt2s = singles.tile([P, KD, N], fp8, tag="zt2s")
            rh1s = singles.tile([P, KD, CH], fp8, tag="rh1s")
            rh2s = singles.tile([P, KD, CH], fp8, tag="rh2s")
            s1s = singles.tile([P, NIB], f32, tag="s1s")
            s2s = singles.tile([P, NIB], f32, tag="s2s")
            acc1 = singles.tile([P, NIB], f32, tag="acc1")
            acc2 = singles.tile([P, NIB], f32, tag="acc2")
            ones = singles.tile([P, 1], bf16, tag="ones")
            nc.vector.memset(ones, 1.0)
            ecol_s = singles.tile([1, CH], f32, tag="ecol_s")
            cs = pscolp.tile([1, CH], f32, tag="cs")

            # ---- input DMAs (rhs + scales first: needed by every ib) ----
            nc.sync.dma_start(out=rh1s, in_=rh1v)
            nc.sync.dma_start(out=rh2s, in_=rh2v)
            nc.sync.dma_start(out=s1s, in_=s1.rearrange("(b p) -> p b", p=P))
            nc.sync.dma_start(out=s2s, in_=s2.rearrange("(b p) -> p b", p=P))
            nc.sync.dma_start(out=zt1s, in_=zt1v)
            nc.sync.dma_start(out=zt2s, in_=zt2v)

            # ---- main ----
            def _main_body():
                esb_pend = []

                def _colsum(item):
                    ibx, esb = item
                    for jt in range(2):
                        nc.tensor.matmul(
                            cs[0:1, jt * 512:(jt + 1) * 512],
                            lhsT=ones,
                            rhs=esb[:, jt * 512:(jt + 1) * 512],
                            start=(ibx == 0),
                            stop=(ibx == NIB - 1),
                            skip_group_check=True,
                        )

                # pass 1: z1 row blocks x [h1c | h2c] -> refl1 + between1
                for ib in range(NIB):
                    psa = psp.tile([P, 1024], f32, tag="ps", name="psa")
                    psb = psp.tile([P, 1024], f32, tag="ps", name="psb")
                    for kp in range(2):
                        lhsT = zt1s[:, 2 * kp:2 * kp + 2, ib * P:(ib + 1) * P]
                        for jt in range(2):
                            nc.tensor.matmul(
                                psa[:, jt * 512:(jt + 1) * 512],
                                lhsT=lhsT,
                                rhs=rh1s[:, 2 * kp:2 * kp + 2,
                                         jt * 512:(jt + 1) * 512],
                                start=(kp == 0), stop=(kp == 1),
                                perf_mode=DR)
                        for jt in range(2):
                            nc.tensor.matmul(
                                psb[:, jt * 512:(jt + 1) * 512],
                                lhsT=lhsT,
                                rhs=rh2s[:, 2 * kp:2 * kp + 2,
                                         jt * 512:(jt + 1) * 512],
                                start=(kp == 0), stop=(kp == 1),
                                perf_mode=DR)
                    esa = esp.tile([P, 1024], bf16, tag="esa", bufs=3)
                    nc.scalar.activation(out=esa, in_=psa, func=AF.Exp,
                                         scale=s1s[:, ib:ib + 1])
                    esb = esp.tile([P, 1024], bf16, tag="esb", bufs=4)
                    nc.scalar.activation(out=esb, in_=psb, func=AF.Exp,
                                         scale=s1s[:, ib:ib + 1])
                    # refl1+between1 rowsums: 2x-rate add, then one 1x reduce
                    esum = esp.tile([P, 1024], bf16, tag="esum", bufs=3)
                    nc.vector.tensor_add(esum, esa, esb)
                    nc.vector.tensor_reduce(acc1[:, ib:ib + 1], esum,
                                            axis=mybir.AxisListType.X,
                                            op=ALU.add)
                    esb_pend.append((ib, esb))
                    # colsum matmuls lag 2 iterations so PE never waits on ACT
                    if len(esb_pend) > 2:
                        _colsum(esb_pend.pop(0))
                for item in esb_pend:
                    _colsum(item)

                # pass 2: z2 row blocks x h2c -> refl2
                for ib in range(NIB):
                    psc = psp.tile([P, 1024], f32, tag="ps", name="psc")
                    for kp in range(2):
                        lhsT = zt2s[:, 2 * kp:2 * kp + 2, ib * P:(ib + 1) * P]
                        for jt in range(2):
                            nc.tensor.matmul(
                                psc[:, jt * 512:(jt + 1) * 512],
                                lhsT=lhsT,
                                rhs=rh2s[:, 2 * kp:2 * kp + 2,
                                         jt * 512:(jt + 1) * 512],
                                start=(kp == 0), stop=(kp == 1),
                                perf_mode=DR)
                    esc = esp.tile([P, 1024], bf16, tag="esa", bufs=3)
                    nc.scalar.activation(out=esc, in_=psc, func=AF.Exp,
                                         scale=s2s[:, ib:ib + 1])
                    nc.vector.tensor_reduce(acc2[:, ib:ib + 1], esc,
                                            axis=mybir.AxisListType.X,
                                            op=ALU.add)

            if loop is not None:
                with tc.For_i(0, loop):
                    _main_body()
            else:
                for _rep in range(repeat):
                    _main_body()

            # stage colsums to SBUF
            nc.vector.tensor_copy(ecol_s, cs)
            nc.sync.dma_start(out=ecol[:].rearrange("(o c) -> o c", o=1),
                              in_=ecol_s)
            nc.sync.dma_start(
                out=partials[0].rearrange("(b p) -> p b", p=P), in_=acc1)
            nc.sync.dma_start(
                out=partials[1].rearrange("(b p) -> p b", p=P), in_=acc2)

    nc.compile()
    return nc


def _get_nc(repeat=1, loop=None):
    key = ("nc", repeat, loop)
    if key not in _CACHE:
        _CACHE[key] = _build(repeat, loop=loop)
    return _CACHE[key]


def _host_prep(z1, z2):
    fp8 = ml_dtypes.float8_e4m3
    z1 = np.asarray(z1, dtype=np.float32)
    z2 = np.asarray(z2, dtype=np.float32)
    n1 = np.maximum(np.linalg.norm(z1, axis=1), 1e-12)
    n2 = np.maximum(np.linalg.norm(z2, axis=1), 1e-12)
    h1 = z1 / n1[:, None]
    h2 = z2 / n2[:, None]
    z1_8 = z1.astype(fp8)
    z2_8 = z2.astype(fp8)
    r1_8 = (h1 * RS).astype(fp8)
    r2_8 = (h2 * RS).astype(fp8)
    s1 = (1.0 / (RS * TAU * n1)).astype(np.float32)
    s2 = (1.0 / (RS * TAU * n2)).astype(np.float32)
    return z1_8, z2_8, r1_8, r2_8, s1, s2, h1, h2, n1, n2


def make_in_maps(z1, z2):
    z1_8, z2_8, r1_8, r2_8, s1, s2, _, _, _, _ = _host_prep(z1, z2)
    zt1 = np.ascontiguousarray(z1_8.T)
    zt2 = np.ascontiguousarray(z2_8.T)
    rt1 = r1_8.T
    rt2 = r2_8.T
    in_maps = []
    for r in range(NCORES):
        in_maps.append({
            "zt1": zt1, "zt2": zt2,
            "rh1": np.ascontiguousarray(rt1[:, r * CH:(r + 1) * CH]),
            "rh2": np.ascontiguousarray(rt2[:, r * CH:(r + 1) * CH]),
            "s1": s1, "s2": s2,
        })
    return in_maps


def kernel(z1, z2):
    from concourse.bass_utils import run_bass_kernel_spmd

    z1_8, z2_8, r1_8, r2_8, s1, s2, h1, h2, n1, n2 = _host_prep(z1, z2)
    in_maps = make_in_maps(z1, z2)

    nc = _get_nc()
    res = run_bass_kernel_spmd(nc, in_maps, core_ids=list(range(NCORES)))

    S1 = np.zeros(N, dtype=np.float64)
    S2 = np.zeros(N, dtype=np.float64)
    for r in range(NCORES):
        out = res.results[r]
        S1 += out["partials"][0].astype(np.float64)
        S2 += out["partials"][1].astype(np.float64)
        S2[r * CH:(r + 1) * CH] += out["ecol"].astype(np.float64)

    # exact diagonal corrections, computed from the same fp8 data the
    # device used: refl_ii = exp(s_i * (z8_i . r8_i))
    q1 = (z1_8.astype(np.float64) * r1_8.astype(np.float64)).sum(1) \
        * s1.astype(np.float64)
    q2 = (z2_8.astype(np.float64) * r2_8.astype(np.float64)).sum(1) \
        * s2.astype(np.float64)
    v5 = (h1.astype(np.float64) * h2.astype(np.float64)).sum(1) / TAU

    loss = 0.5 * (np.log(S1 - np.exp(q1)) + np.log(S2 - np.exp(q2))) - v5
    return np.float32(loss.sum())
